# revision 4
# baseline (speedup 1.0000x reference)
"""Trainium2 Bass kernel: segment-mean over contextual encodings.

Reference computation:
    emb  = concat([x[:, 257:769, :], broadcast(x[:, 0:1, :])], -1)   # [B, S, 2D]
    out  = scatter_mean(emb by segment_ids[:, 257:769]) -> [2048, 2D]

Sharding strategy: shard the OUTPUT segments across the 8 cores (256
segments each) so no collective is needed. Host-side prep partitions and
packs the token rows by owning (core, 128-seg bucket) into one contiguous
bf16 block per core, so the device does pure sequential HWDGE streaming —
no indirect gather, no GpSimd/Pool engine at all.

Algebraic split: output columns [0:1024] are the segment-sum of x-window
rows (memory-bound, one-hot matmul accumulation); columns [1024:2048] are
the broadcast CLS row, whose segment-sum factorizes as per-(segment,batch)
counts @ x[:,0,:] — a tiny [128,32]@[32,1024] matmul per bucket. Counts and
reciprocals come from segment_ids alone (metadata) and are precomputed on
the host.

Per core: stream ~18 chunks of 128 rows x 1024 bf16 (~4.5 MiB) via a few
large sync DMAs, build 128-wide one-hot matrices on DVE, accumulate
segment sums with bf16 matmuls in PSUM, scale by host-provided
reciprocals, and write the 256-row output slice (inputs on the Sync HWDGE
queue, outputs on the Activation HWDGE queue so the streams interleave).
"""

import numpy as np

B = 32          # batch
TSEQ = 1024     # sequence length of x
D = 1024        # feature dim
SENT = 512
CTX = 256
NSEG = 2048
LO = 1 + CTX    # 257
HI = LO + SENT  # 769
NCORES = 8
SEGS_PER_CORE = NSEG // NCORES   # 256
P = 128
BUCKETS = SEGS_PER_CORE // P     # 2

LAST_RESULTS = None  # BassKernelResults of the most recent run (for test.py)


def _build_shards(xw32, x0_32, seg_flat):
    """Pack rows by owning (core, bucket): per core a [P, NCH*D] bf16 block
    in chunk-major layout plus local-segment labels, counts, reciprocals."""
    import ml_dtypes

    bf16 = ml_dtypes.bfloat16
    tok = np.nonzero(seg_flat >= 0)[0]
    tseg = seg_flat[tok]
    core_id = tseg // SEGS_PER_CORE
    bucket_id = (tseg % SEGS_PER_CORE) // P
    local_id = (tseg % P).astype(np.float32)
    batch_id = tok // SENT

    counts = np.zeros((NCORES, BUCKETS), np.int64)
    for c in range(NCORES):
        sel = core_id == c
        for b in range(BUCKETS):
            counts[c, b] = int(np.sum(sel & (bucket_id == b)))
    CB = max(1, int(-(-counts.max() // P)))    # chunks per bucket
    NCH = BUCKETS * CB

    xw16 = xw32.astype(bf16)
    xs = np.zeros((NCORES, P, NCH * D), bf16)
    segl = np.full((NCORES, P, NCH), -1.0, np.float32)
    cmT = np.zeros((NCORES, B, BUCKETS * P), np.float32)
    recip = np.ones((NCORES, P, BUCKETS), np.float32)
    for c in range(NCORES):
        for b in range(BUCKETS):
            m = (core_id == c) & (bucket_id == b)
            rows = tok[m]
            loc = local_id[m]
            bat = batch_id[m]
            n = rows.size
            npad = CB * P
            data = np.zeros((npad, D), bf16)
            data[:n] = xw16[rows]
            lab = np.full(npad, -1.0, np.float32)
            lab[:n] = loc
            # chunk-major -> partition-major: chunk k row p = packed row k*P+p
            xs[c, :, b * CB * D:(b + 1) * CB * D] = (
                data.reshape(CB, P, D).transpose(1, 0, 2).reshape(P, CB * D))
            segl[c, :, b * CB:(b + 1) * CB] = lab.reshape(CB, P).T
            np.add.at(cmT[c], (bat, b * P + loc.astype(np.int64)), 1.0)
            tot = np.bincount(loc.astype(np.int64), minlength=P)
            recip[c, :, b] = 1.0 / np.maximum(tot, 1.0)
    return CB, xs, segl, cmT.astype(bf16), recip, x0_32.astype(bf16)


def _build_program(CB):
    import concourse.bacc as bacc
    import concourse.tile as tile
    from concourse import mybir

    f32 = mybir.dt.float32
    bf16 = mybir.dt.bfloat16
    NCH = BUCKETS * CB
    GSZ = 3                                  # chunks per input DMA group
    groups = [(g, min(g + GSZ, NCH)) for g in range(0, NCH, GSZ)]

    nc = bacc.Bacc("TRN2", target_bir_lowering=False, debug=False,
                   num_devices=NCORES)
    xs_d = nc.dram_tensor("xs", [P, NCH * D], bf16, kind="ExternalInput")
    segl_d = nc.dram_tensor("segl", [P, NCH], f32, kind="ExternalInput")
    iota_d = nc.dram_tensor("iota", [P, P], f32, kind="ExternalInput")
    x0_d = nc.dram_tensor("x0", [B, D], bf16, kind="ExternalInput")
    cmT_d = nc.dram_tensor("cmT", [B, BUCKETS * P], bf16,
                           kind="ExternalInput")
    recip_d = nc.dram_tensor("recip", [P, BUCKETS], f32,
                             kind="ExternalInput")
    out_d = nc.dram_tensor("out", [SEGS_PER_CORE, 2 * D], f32,
                           kind="ExternalOutput")

    with tile.TileContext(nc) as tc:
        with (
            tc.tile_pool(name="const", bufs=1) as constp,
            tc.tile_pool(name="data", bufs=len(groups)) as datap,
            tc.tile_pool(name="oh", bufs=4) as ohp,
            tc.tile_pool(name="outs", bufs=4) as outsp,
            tc.tile_pool(name="psum", bufs=2, space="PSUM") as psump,
        ):
            # small metadata first so the one-hot pipeline starts immediately
            segl_sb = constp.tile([P, NCH], f32)
            nc.sync.dma_start(out=segl_sb[:], in_=segl_d.ap()[:])
            iota_sb = constp.tile([P, P], f32)
            nc.sync.dma_start(out=iota_sb[:], in_=iota_d.ap()[:])
            recip_sb = constp.tile([P, BUCKETS], f32)
            nc.sync.dma_start(out=recip_sb[:], in_=recip_d.ap()[:])
            x0_sb = constp.tile([B, D], bf16)
            nc.sync.dma_start(out=x0_sb[:], in_=x0_d.ap()[:])
            cmT_sb = constp.tile([B, BUCKETS * P], bf16)
            nc.sync.dma_start(out=cmT_sb[:], in_=cmT_d.ap()[:])

            # the input stream: one tile per DMA group, issued up-front so
            # the Sync queue drains back-to-back at line rate
            gdata = []
            for gi, (g0, g1) in enumerate(groups):
                dt_g = datap.tile([P, (g1 - g0) * D], bf16, tag="data",
                                  name=f"g{gi}")
                nc.sync.dma_start(out=dt_g[:],
                                  in_=xs_d.ap()[:, g0 * D:g1 * D])
                gdata.append(dt_g)

            # one "acc" tag, bufs=2: acc0/acc1 hold the two rotating buffers;
            # each bucket's cls_ps reuses its acc's buffer after the final
            # read (the o1 scale), keeping total PSUM at 4 banks
            acc = [psump.tile([P, D], f32, tag="acc", name=f"acc{i}")
                   for i in range(BUCKETS)]

            for ci in range(NCH):
                b = ci // CB
                first = (ci % CB) == 0
                last = (ci % CB) == CB - 1
                gi, col = divmod(ci, GSZ)

                oh = ohp.tile([P, P], bf16, tag="oh")
                nc.vector.tensor_tensor(
                    out=oh[:], in0=iota_sb[:],
                    in1=segl_sb[:, ci:ci + 1].to_broadcast([P, P]),
                    op=mybir.AluOpType.is_equal)
                for j in range(2):
                    nc.tensor.matmul(
                        out=acc[b][:, j * 512:(j + 1) * 512],
                        lhsT=oh[:],
                        rhs=gdata[gi][:, col * D + j * 512:
                                      col * D + (j + 1) * 512],
                        start=first, stop=last)

                if last:
                    # bucket epilogue: scale by 1/count, write out; outputs
                    # go on the ACT HWDGE queue so they interleave with the
                    # Sync input stream instead of queueing behind it
                    o1 = outsp.tile([P, D], f32, tag="o")
                    nc.vector.tensor_scalar_mul(
                        out=o1[:], in0=acc[b][:],
                        scalar1=recip_sb[:, b:b + 1])
                    nc.scalar.dma_start(
                        out=out_d.ap()[b * P:(b + 1) * P, 0:D], in_=o1[:])

                    cls_ps = psump.tile([P, D], f32, tag="acc")
                    for j in range(2):
                        nc.tensor.matmul(
                            out=cls_ps[:, j * 512:(j + 1) * 512],
                            lhsT=cmT_sb[:, b * P:(b + 1) * P],
                            rhs=x0_sb[:, j * 512:(j + 1) * 512],
                            start=True, stop=True)
                    o2 = outsp.tile([P, D], f32, tag="o")
                    nc.scalar.activation(
                        out=o2[:], in_=cls_ps[:],
                        func=mybir.ActivationFunctionType.Copy,
                        scale=recip_sb[:, b:b + 1])
                    nc.scalar.dma_start(
                        out=out_d.ap()[b * P:(b + 1) * P, D:2 * D],
                        in_=o2[:])

    nc.compile()
    return nc


def kernel(x, segment_ids):
    global LAST_RESULTS
    from concourse.bass_utils import run_bass_kernel_spmd

    x = np.asarray(x, dtype=np.float32)
    seg_all = np.asarray(segment_ids).astype(np.int64)
    assert x.shape == (B, TSEQ, D), x.shape
    assert seg_all.shape == (B, TSEQ), seg_all.shape

    xw = np.ascontiguousarray(x[:, LO:HI, :].reshape(B * SENT, D))
    x0 = np.ascontiguousarray(x[:, 0, :])
    seg_flat = seg_all[:, LO:HI].reshape(-1)

    CB, xs, segl, cmT, recip, x0_16 = _build_shards(xw, x0, seg_flat)
    nc = _build_program(CB)

    iota = np.broadcast_to(np.arange(P, dtype=np.float32), (P, P))
    iota = np.ascontiguousarray(iota)
    in_maps = [
        {"xs": xs[c], "segl": segl[c], "iota": iota, "x0": x0_16,
         "cmT": cmT[c], "recip": recip[c]}
        for c in range(NCORES)
    ]
    last_err = None
    for _attempt in range(3):
        try:
            res = run_bass_kernel_spmd(nc, in_maps, list(range(NCORES)))
            break
        except Exception as e:  # transient NRT device errors happen; retry
            last_err = e
    else:
        raise last_err
    LAST_RESULTS = res
    return np.concatenate([res.results[c]["out"] for c in range(NCORES)],
                          axis=0)


# revision 6
# speedup vs baseline: 1.0254x; 1.0254x over previous
"""Trainium2 Bass kernel: segment-mean over contextual encodings.

Reference computation:
    emb  = concat([x[:, 257:769, :], broadcast(x[:, 0:1, :])], -1)   # [B, S, 2D]
    out  = scatter_mean(emb by segment_ids[:, 257:769]) -> [2048, 2D]

Sharding strategy: shard the OUTPUT segments across the 8 cores (256
segments each) so no collective is needed. Host-side prep partitions and
packs the token rows by owning (core, 128-seg bucket) into one contiguous
bf16 block per core, so the device does pure sequential HWDGE streaming —
no indirect gather, no GpSimd/Pool engine at all.

Algebraic split: output columns [0:1024] are the segment-sum of x-window
rows (memory-bound, one-hot matmul accumulation); columns [1024:2048] are
the broadcast CLS row, whose segment-sum factorizes as per-(segment,batch)
counts @ x[:,0,:] — a tiny [128,32]@[32,1024] matmul per bucket. Counts and
reciprocals come from segment_ids alone (metadata) and are precomputed on
the host.

Per core: stream ~18 chunks of 128 rows x 1024 bf16 (~4.5 MiB) via a few
large sync DMAs, build 128-wide one-hot matrices on DVE, accumulate
segment sums with bf16 matmuls in PSUM, scale by host-provided
reciprocals, and write the 256-row output slice (inputs on the Sync HWDGE
queue, outputs on the Activation HWDGE queue so the streams interleave).
"""

import numpy as np

B = 32          # batch
TSEQ = 1024     # sequence length of x
D = 1024        # feature dim
SENT = 512
CTX = 256
NSEG = 2048
LO = 1 + CTX    # 257
HI = LO + SENT  # 769
NCORES = 8
SEGS_PER_CORE = NSEG // NCORES   # 256
P = 128
BUCKETS = SEGS_PER_CORE // P     # 2

LAST_RESULTS = None  # BassKernelResults of the most recent run (for test.py)


def _build_shards(xw32, x0_32, seg_flat):
    """Pack rows by owning (core, bucket): per core a [P, NCH*D] bf16 block
    in chunk-major layout plus local-segment labels, counts, reciprocals."""
    import ml_dtypes

    bf16 = ml_dtypes.bfloat16
    tok = np.nonzero(seg_flat >= 0)[0]
    tseg = seg_flat[tok]
    core_id = tseg // SEGS_PER_CORE
    bucket_id = (tseg % SEGS_PER_CORE) // P
    local_id = (tseg % P).astype(np.float32)
    batch_id = tok // SENT

    counts = np.zeros((NCORES, BUCKETS), np.int64)
    for c in range(NCORES):
        sel = core_id == c
        for b in range(BUCKETS):
            counts[c, b] = int(np.sum(sel & (bucket_id == b)))
    CB = max(1, int(-(-counts.max() // P)))    # chunks per bucket
    NCH = BUCKETS * CB

    xw16 = xw32.astype(bf16)
    xs = np.zeros((NCORES, P, NCH * D), bf16)
    segl = np.full((NCORES, P, NCH), -1.0, np.float32)
    cmT = np.zeros((NCORES, B, BUCKETS * P), np.float32)
    recip = np.ones((NCORES, P, BUCKETS), np.float32)
    for c in range(NCORES):
        for b in range(BUCKETS):
            m = (core_id == c) & (bucket_id == b)
            rows = tok[m]
            loc = local_id[m]
            bat = batch_id[m]
            n = rows.size
            npad = CB * P
            data = np.zeros((npad, D), bf16)
            data[:n] = xw16[rows]
            lab = np.full(npad, -1.0, np.float32)
            lab[:n] = loc
            # chunk-major -> partition-major: chunk k row p = packed row k*P+p
            xs[c, :, b * CB * D:(b + 1) * CB * D] = (
                data.reshape(CB, P, D).transpose(1, 0, 2).reshape(P, CB * D))
            segl[c, :, b * CB:(b + 1) * CB] = lab.reshape(CB, P).T
            np.add.at(cmT[c], (bat, b * P + loc.astype(np.int64)), 1.0)
            tot = np.bincount(loc.astype(np.int64), minlength=P)
            recip[c, :, b] = 1.0 / np.maximum(tot, 1.0)
    return CB, xs, segl, cmT.astype(bf16), recip, x0_32.astype(bf16)


def _build_program(CB):
    import concourse.bacc as bacc
    import concourse.tile as tile
    from concourse import mybir

    f32 = mybir.dt.float32
    bf16 = mybir.dt.bfloat16
    NCH = BUCKETS * CB
    GSZ = 3                                  # chunks per input DMA group
    groups = [(g, min(g + GSZ, NCH)) for g in range(0, NCH, GSZ)]

    nc = bacc.Bacc("TRN2", target_bir_lowering=False, debug=False,
                   num_devices=NCORES)
    xs_d = nc.dram_tensor("xs", [P, NCH * D], bf16, kind="ExternalInput")
    segl_d = nc.dram_tensor("segl", [P, NCH], f32, kind="ExternalInput")
    iota_d = nc.dram_tensor("iota", [P, P], f32, kind="ExternalInput")
    x0_d = nc.dram_tensor("x0", [B, D], bf16, kind="ExternalInput")
    cmT_d = nc.dram_tensor("cmT", [B, BUCKETS * P], bf16,
                           kind="ExternalInput")
    recip_d = nc.dram_tensor("recip", [P, BUCKETS], f32,
                             kind="ExternalInput")
    out_d = nc.dram_tensor("out", [SEGS_PER_CORE, 2 * D], f32,
                           kind="ExternalOutput")

    with tile.TileContext(nc) as tc:
        with (
            tc.tile_pool(name="const", bufs=1) as constp,
            tc.tile_pool(name="data", bufs=len(groups)) as datap,
            tc.tile_pool(name="oh", bufs=4) as ohp,
            tc.tile_pool(name="outs", bufs=4) as outsp,
            tc.tile_pool(name="psum", bufs=2, space="PSUM") as psump,
        ):
            # metadata on the ACT HWDGE queue so the Sync queue is free to
            # start the bulk data stream immediately
            segl_sb = constp.tile([P, NCH], f32)
            nc.scalar.dma_start(out=segl_sb[:], in_=segl_d.ap()[:])
            iota_sb = constp.tile([P, P], f32)
            nc.scalar.dma_start(out=iota_sb[:], in_=iota_d.ap()[:])
            recip_sb = constp.tile([P, BUCKETS], f32)
            nc.scalar.dma_start(out=recip_sb[:], in_=recip_d.ap()[:])
            x0_sb = constp.tile([B, D], bf16)
            nc.scalar.dma_start(out=x0_sb[:], in_=x0_d.ap()[:])
            cmT_sb = constp.tile([B, BUCKETS * P], bf16)
            nc.scalar.dma_start(out=cmT_sb[:], in_=cmT_d.ap()[:])

            # the input stream: one tile per DMA group, issued up-front so
            # the Sync queue drains back-to-back at line rate
            gdata = []
            for gi, (g0, g1) in enumerate(groups):
                dt_g = datap.tile([P, (g1 - g0) * D], bf16, tag="data",
                                  name=f"g{gi}")
                nc.sync.dma_start(out=dt_g[:],
                                  in_=xs_d.ap()[:, g0 * D:g1 * D])
                gdata.append(dt_g)

            # one "acc" tag, bufs=2: acc0/acc1 hold the two rotating buffers;
            # each bucket's cls_ps reuses its acc's buffer after the final
            # read (the o1 scale), keeping total PSUM at 4 banks
            acc = [psump.tile([P, D], f32, tag="acc", name=f"acc{i}")
                   for i in range(BUCKETS)]

            # one-hot matrices built one DVE op per group (fewer sem waits)
            ohg = []
            for gi, (g0, g1) in enumerate(groups):
                n = g1 - g0
                oh_all = ohp.tile([P, n * P], bf16, tag="oh")
                nc.vector.tensor_tensor(
                    out=oh_all[:].rearrange("p (g q) -> p g q", g=n),
                    in0=iota_sb[:].unsqueeze(1).to_broadcast([P, n, P]),
                    in1=segl_sb[:, g0:g1].unsqueeze(2).to_broadcast(
                        [P, n, P]),
                    op=mybir.AluOpType.is_equal)
                ohg.append(oh_all)

            for ci in range(NCH):
                b = ci // CB
                first = (ci % CB) == 0
                last = (ci % CB) == CB - 1
                gi, col = divmod(ci, GSZ)

                for j in range(2):
                    nc.tensor.matmul(
                        out=acc[b][:, j * 512:(j + 1) * 512],
                        lhsT=ohg[gi][:, col * P:(col + 1) * P],
                        rhs=gdata[gi][:, col * D + j * 512:
                                      col * D + (j + 1) * 512],
                        start=first, stop=last)

                if last:
                    # bucket epilogue: scale by 1/count, write out; outputs
                    # go on the ACT HWDGE queue so they interleave with the
                    # Sync input stream instead of queueing behind it
                    o1 = outsp.tile([P, D], f32, tag="o")
                    nc.vector.tensor_scalar_mul(
                        out=o1[:], in0=acc[b][:],
                        scalar1=recip_sb[:, b:b + 1])
                    nc.scalar.dma_start(
                        out=out_d.ap()[b * P:(b + 1) * P, 0:D], in_=o1[:])

                    cls_ps = psump.tile([P, D], f32, tag="acc")
                    for j in range(2):
                        nc.tensor.matmul(
                            out=cls_ps[:, j * 512:(j + 1) * 512],
                            lhsT=cmT_sb[:, b * P:(b + 1) * P],
                            rhs=x0_sb[:, j * 512:(j + 1) * 512],
                            start=True, stop=True)
                    o2 = outsp.tile([P, D], f32, tag="o")
                    nc.scalar.activation(
                        out=o2[:], in_=cls_ps[:],
                        func=mybir.ActivationFunctionType.Copy,
                        scale=recip_sb[:, b:b + 1])
                    nc.scalar.dma_start(
                        out=out_d.ap()[b * P:(b + 1) * P, D:2 * D],
                        in_=o2[:])

    nc.compile()
    return nc


def kernel(x, segment_ids):
    global LAST_RESULTS
    from concourse.bass_utils import run_bass_kernel_spmd

    x = np.asarray(x, dtype=np.float32)
    seg_all = np.asarray(segment_ids).astype(np.int64)
    assert x.shape == (B, TSEQ, D), x.shape
    assert seg_all.shape == (B, TSEQ), seg_all.shape

    xw = np.ascontiguousarray(x[:, LO:HI, :].reshape(B * SENT, D))
    x0 = np.ascontiguousarray(x[:, 0, :])
    seg_flat = seg_all[:, LO:HI].reshape(-1)

    CB, xs, segl, cmT, recip, x0_16 = _build_shards(xw, x0, seg_flat)
    nc = _build_program(CB)

    iota = np.broadcast_to(np.arange(P, dtype=np.float32), (P, P))
    iota = np.ascontiguousarray(iota)
    in_maps = [
        {"xs": xs[c], "segl": segl[c], "iota": iota, "x0": x0_16,
         "cmT": cmT[c], "recip": recip[c]}
        for c in range(NCORES)
    ]
    last_err = None
    for _attempt in range(3):
        try:
            res = run_bass_kernel_spmd(nc, in_maps, list(range(NCORES)))
            break
        except Exception as e:  # transient NRT device errors happen; retry
            last_err = e
    else:
        raise last_err
    LAST_RESULTS = res
    return np.concatenate([res.results[c]["out"] for c in range(NCORES)],
                          axis=0)
